# revision 26
# baseline (speedup 1.0000x reference)
"""NeuralRuleEnsemble forward kernel for 8x Trainium2 NeuronCores.

Reference computation (N=100000 rows, D=256 features, R=128 rules, P=4
propositions/rule, K=8 feature-subset size):

    xg     = x[:, indices]                                   # [N, R, K]
    scores = relu(einsum('nrk,rpk->nrp', xg, weights) + biases)
    out    = sum_r c[r] * min_p scores[:, r, :]              # [N, 1]

Reformulation used here: the gather + tiny per-rule matmuls are one dense
matmul against a scatter-accumulated weight matrix

    W_dense[d, r*4+p] = sum_{k: indices[r,k]==d} weights[r,p,k]
    z[n, rp] = x @ W_dense + biases_flat                     # [N, 512]

(exact equivalence, including duplicate indices).  On-chip, per 512-row
chunk, z^T is computed p-major as four [128r, 512n] PSUM tiles (bias folded
in via a K=1 ones-row matmul), the min over p is a 3-op DVE min tree
(min/relu commute so relu is applied once after the min), and the final
sum_r c[r] * (.) is one more matmul with c as the stationary vector.

Data-parallel over rows: each of the 8 cores gets 12500 rows (padded to
12800).  x is transposed on the host so the contraction dim d lands on SBUF
partitions with no on-chip transpose.  Matmuls run in float32r (TF32 mode,
1 cycle/row for free dim >= 256 vs 4 for fp32; ~5e-4 element error).
"""
import sys
import numpy as np

for _p in ("/opt/trn_rl_repo",):
    if _p not in sys.path:
        sys.path.append(_p)

N, D = 100000, 256
R, P, K = 128, 4, 8
N_CORES = 8
N_PER_CORE = 12800          # 12500 rounded up to a multiple of 512
CHUNK = 512                 # matmul moving-dim chunk (one PSUM bank of fp32)
# DMA superchunk sizes (columns): small first chunks so the PE starts
# almost immediately instead of waiting for one huge 2.5 MB transfer.
SUPERS = [512, 1024, 2048, 2560, 2560, 2560, 1024, 512]
assert sum(SUPERS) == N_PER_CORE and all(s % CHUNK == 0 for s in SUPERS)

_compiled = None


def _build_program():
    import concourse.mybir as mybir
    from concourse import bacc
    from concourse.tile import TileContext

    f32 = mybir.dt.float32
    f32r = mybir.dt.float32r
    Alu = mybir.AluOpType
    Act = mybir.ActivationFunctionType

    nc = bacc.Bacc(num_devices=N_CORES)
    xt = nc.dram_tensor("xt", [2, 128, N_PER_CORE], f32r, kind="ExternalInput")
    wts = nc.dram_tensor("wts", [128, 2, P, 128], f32r, kind="ExternalInput")
    # biases p-major [128 r, 4 p] fp32 + c [128, 1] f32r
    bia = nc.dram_tensor("bia", [128, P], f32, kind="ExternalInput")
    cl = nc.dram_tensor("cl", [128, 1], f32r, kind="ExternalInput")
    out = nc.dram_tensor("out", [1, N_PER_CORE], f32, kind="ExternalOutput")

    with TileContext(nc) as tc:
        with tc.tile_pool(name="consts", bufs=1) as consts, \
             tc.tile_pool(name="xin", bufs=3) as xin, \
             tc.tile_pool(name="mins", bufs=3) as mins, \
             tc.tile_pool(name="orows", bufs=2) as orows, \
             tc.tile_pool(name="ps", bufs=6, space="PSUM") as ps, \
             tc.tile_pool(name="ops", bufs=2, space="PSUM") as ops:

            # const loads go out on the ACT engine's HWDGE path so their
            # descriptor triggers don't serialize behind the x loads on SP
            wt_sb = consts.tile([128, 2, P, 128], f32r)
            # split by d-chunk so the first (dc=0) matmuls only wait 256KB
            nc.scalar.dma_start(out=wt_sb[:, 0], in_=wts[:, 0])
            nc.scalar.dma_start(out=wt_sb[:, 1], in_=wts[:, 1])
            b_sb = consts.tile([128, P], f32)
            nc.scalar.dma_start(out=b_sb, in_=bia[:, :])
            c_sb = consts.tile([128, 1], f32r)
            nc.scalar.dma_start(out=c_sb, in_=cl[:, :])

            # the c-matmul + out-copy of chunk j are emitted after chunk
            # j+1's W-matmuls so the PE never stalls on the min-chain
            pending = []

            def flush_pending():
                while pending:
                    mr_, orow_, nsl_ = pending.pop(0)
                    o_ps = ops.tile([1, CHUNK], f32, tag="o_ps")
                    nc.tensor.matmul(o_ps, c_sb, mr_, start=True, stop=True)
                    nc.scalar.copy(out=orow_[:, nsl_], in_=o_ps)

            n0 = 0
            for sc, sup in enumerate(SUPERS):
                x_sb = xin.tile([128, 2, sup], f32r, tag="x_sb")
                if sc == 0:
                    # split by d-chunk: lets the very first matmuls start
                    # after half the transfer
                    for dc in range(2):
                        nc.sync.dma_start(
                            out=x_sb[:, dc],
                            in_=xt[dc, :, n0:n0 + sup])
                else:
                    nc.sync.dma_start(
                        out=x_sb,
                        in_=xt[:, :, n0:n0 + sup].rearrange(
                            "dc d n -> d dc n"),
                    )
                orow = orows.tile([1, sup], f32, tag="orow")

                for j in range(sup // CHUNK):
                    is_last = (sc == len(SUPERS) - 1 and
                               j == sup // CHUNK - 1)
                    nsl = slice(j * CHUNK, (j + 1) * CHUNK)
                    pst = []
                    for p in range(P):
                        z = ps.tile([128, CHUNK], f32, tag="z")
                        nc.tensor.matmul(z, wt_sb[:, 0, p, :], x_sb[:, 0, nsl],
                                         start=True, stop=False)
                        nc.tensor.matmul(z, wt_sb[:, 1, p, :], x_sb[:, 1, nsl],
                                         start=False, stop=True)
                        pst.append(z)

                    # min_p (z_p + b_p): ACT seeds with a bias-add copy out
                    # of PSUM, DVE folds each remaining PSUM tile via
                    # (ps + b) min acc; relu once at the end (commutes with
                    # min).  The last chunk uses two independent pair-chains
                    # instead -- lower latency, shorter kernel tail.
                    a0 = mins.tile([128, CHUNK], f32, tag="a0")
                    nc.scalar.activation(out=a0, in_=pst[0], func=Act.Identity,
                                         bias=b_sb[:, 0:1])
                    a1 = mins.tile([128, CHUNK], f32, tag="a1")
                    nc.vector.scalar_tensor_tensor(
                        out=a1, in0=pst[1], scalar=b_sb[:, 1:2], in1=a0,
                        op0=Alu.add, op1=Alu.min)
                    if not is_last:
                        a2 = mins.tile([128, CHUNK], f32, tag="a2")
                        nc.vector.scalar_tensor_tensor(
                            out=a2, in0=pst[2], scalar=b_sb[:, 2:3], in1=a1,
                            op0=Alu.add, op1=Alu.min)
                        a3 = mins.tile([128, CHUNK], f32, tag="a3")
                        nc.vector.scalar_tensor_tensor(
                            out=a3, in0=pst[3], scalar=b_sb[:, 3:4], in1=a2,
                            op0=Alu.add, op1=Alu.min)
                        mr = mins.tile([128, CHUNK], f32r, tag="mr")
                        nc.scalar.activation(out=mr, in_=a3, func=Act.Relu)
                    else:
                        a2 = mins.tile([128, CHUNK], f32, tag="a2")
                        nc.scalar.activation(out=a2, in_=pst[2],
                                             func=Act.Identity,
                                             bias=b_sb[:, 2:3])
                        a3 = mins.tile([128, CHUNK], f32, tag="a3")
                        nc.vector.scalar_tensor_tensor(
                            out=a3, in0=pst[3], scalar=b_sb[:, 3:4], in1=a2,
                            op0=Alu.add, op1=Alu.min)
                        ff = mins.tile([128, CHUNK], f32, tag="ff")
                        nc.vector.tensor_tensor(out=ff, in0=a1, in1=a3,
                                                op=Alu.min)
                        mr = mins.tile([128, CHUNK], f32r, tag="mr")
                        nc.scalar.activation(out=mr, in_=ff, func=Act.Relu)

                    pending.append((mr, orow, nsl))
                    flush_pending()

                # store via SWDGE (Pool queue): its sem wait must not block
                # later x-load triggers in the SP HWDGE FIFO
                nc.gpsimd.dma_start(out=out[:, n0:n0 + sup], in_=orow)
                n0 += sup

    nc.compile()
    return nc


def _get_compiled():
    global _compiled
    if _compiled is None:
        _compiled = _build_program()
    return _compiled


def _host_prep(x, weights, biases, c, indices):
    x = np.ascontiguousarray(np.asarray(x, dtype=np.float32))
    weights = np.asarray(weights, dtype=np.float32)
    biases = np.asarray(biases, dtype=np.float32)
    c = np.asarray(c, dtype=np.float32)
    indices = np.asarray(indices)

    # W_dense[d, r*P+p] = sum_{k: idx[r,k]==d} w[r,p,k]
    w_dense = np.zeros((D, R * P), dtype=np.float32)
    r_idx = np.broadcast_to(np.arange(R)[:, None, None], (R, P, K))
    p_idx = np.broadcast_to(np.arange(P)[None, :, None], (R, P, K))
    d_idx = np.broadcast_to(indices[:, None, :], (R, P, K))
    np.add.at(w_dense, (d_idx.ravel(), (r_idx * P + p_idx).ravel()),
              weights.ravel())

    # [128 d, 2 dc, 4 p, 128 r]
    wts = np.ascontiguousarray(
        w_dense.reshape(2, 128, R, P).transpose(1, 0, 3, 2))

    bia = np.ascontiguousarray(biases)      # [128 r, 4 p]
    cl = np.ascontiguousarray(c.reshape(128, 1))

    xt_pad = np.zeros((D, N_CORES * N_PER_CORE), dtype=np.float32)
    # place each core's 12500 rows at its padded offset
    xsplit = x.T.reshape(D, N_CORES, N // N_CORES)
    xt_pad_v = xt_pad.reshape(D, N_CORES, N_PER_CORE)
    xt_pad_v[:, :, :N // N_CORES] = xsplit

    in_maps = []
    for core in range(N_CORES):
        xt_c = np.ascontiguousarray(
            xt_pad_v[:, core, :].reshape(2, 128, N_PER_CORE))
        in_maps.append({"xt": xt_c, "wts": wts, "bia": bia, "cl": cl})
    return in_maps


def kernel(x, weights, biases, c, indices, _trace=False):
    from concourse.bass_utils import run_bass_kernel_spmd

    nc = _get_compiled()
    in_maps = _host_prep(x, weights, biases, c, indices)
    res = run_bass_kernel_spmd(nc, in_maps, core_ids=list(range(N_CORES)),
                               trace=_trace)
    per_core = np.stack([r["out"].reshape(N_PER_CORE)[:N // N_CORES]
                         for r in res.results])
    out = per_core.reshape(N).astype(np.float32)[:, None]
    if _trace:
        kernel.last_exec_time_ns = res.exec_time_ns
        kernel.last_results = res
    return out


# revision 31
# speedup vs baseline: 1.0509x; 1.0509x over previous
"""NeuralRuleEnsemble forward kernel for 8x Trainium2 NeuronCores.

Reference computation (N=100000 rows, D=256 features, R=128 rules, P=4
propositions/rule, K=8 feature-subset size):

    xg     = x[:, indices]                                   # [N, R, K]
    scores = relu(einsum('nrk,rpk->nrp', xg, weights) + biases)
    out    = sum_r c[r] * min_p scores[:, r, :]              # [N, 1]

Reformulation used here: the gather + tiny per-rule matmuls are one dense
matmul against a scatter-accumulated weight matrix

    W_dense[d, r*4+p] = sum_{k: indices[r,k]==d} weights[r,p,k]
    z[n, rp] = x @ W_dense + biases_flat                     # [N, 512]

(exact equivalence, including duplicate indices).  On-chip, per 512-row
chunk, z^T is computed p-major as four [128r, 512n] PSUM tiles (bias folded
in via a K=1 ones-row matmul), the min over p is a 3-op DVE min tree
(min/relu commute so relu is applied once after the min), and the final
sum_r c[r] * (.) is one more matmul with c as the stationary vector.

Data-parallel over rows: each of the 8 cores gets 12500 rows (padded to
12800).  x is transposed on the host so the contraction dim d lands on SBUF
partitions with no on-chip transpose.  Matmuls run in float32r (TF32 mode,
1 cycle/row for free dim >= 256 vs 4 for fp32; ~5e-4 element error).
"""
import sys
import numpy as np

for _p in ("/opt/trn_rl_repo",):
    if _p not in sys.path:
        sys.path.append(_p)

N, D = 100000, 256
R, P, K = 128, 4, 8
N_CORES = 8
N_PER_CORE = 12800          # 12500 rounded up to a multiple of 512
CHUNK = 512                 # matmul moving-dim chunk (one PSUM bank of fp32)
# DMA superchunk sizes (columns): small first chunks so the PE starts
# almost immediately instead of waiting for one huge 2.5 MB transfer.
SUPERS = [512, 1024, 1536, 2560, 2560, 2560, 1536, 512]
assert sum(SUPERS) == N_PER_CORE and all(s % CHUNK == 0 for s in SUPERS)

_compiled = None


def _build_program():
    import concourse.mybir as mybir
    from concourse import bacc
    from concourse.tile import TileContext

    f32 = mybir.dt.float32
    f32r = mybir.dt.float32r
    Alu = mybir.AluOpType
    Act = mybir.ActivationFunctionType

    nc = bacc.Bacc(num_devices=N_CORES)
    xt = nc.dram_tensor("xt", [2, 128, N_PER_CORE], f32r, kind="ExternalInput")
    wts = nc.dram_tensor("wts", [128, 2, P, 128], f32r, kind="ExternalInput")
    # biases p-major [128 r, 4 p] fp32 + c [128, 1] f32r
    bia = nc.dram_tensor("bia", [128, P], f32, kind="ExternalInput")
    cl = nc.dram_tensor("cl", [128, 1], f32r, kind="ExternalInput")
    out = nc.dram_tensor("out", [1, N_PER_CORE], f32, kind="ExternalOutput")

    with TileContext(nc) as tc:
        with tc.tile_pool(name="consts", bufs=1) as consts, \
             tc.tile_pool(name="xin", bufs=3) as xin, \
             tc.tile_pool(name="mins", bufs=3) as mins, \
             tc.tile_pool(name="orows", bufs=2) as orows, \
             tc.tile_pool(name="ps", bufs=6, space="PSUM") as ps, \
             tc.tile_pool(name="ops", bufs=2, space="PSUM") as ops:

            # const loads go out on the ACT engine's HWDGE path so their
            # descriptor triggers don't serialize behind the x loads on SP
            wt_sb = consts.tile([128, 2, P, 128], f32r)
            # split by d-chunk so the first (dc=0) matmuls only wait 256KB
            nc.scalar.dma_start(out=wt_sb[:, 0], in_=wts[:, 0])
            nc.scalar.dma_start(out=wt_sb[:, 1], in_=wts[:, 1])
            b_sb = consts.tile([128, P], f32)
            nc.gpsimd.dma_start(out=b_sb, in_=bia[:, :])
            c_sb = consts.tile([128, 1], f32r)
            nc.gpsimd.dma_start(out=c_sb, in_=cl[:, :])

            # pre-warm the PE during the initial DMA wait: dependency-free
            # dummy matmuls keep the HAM clock-gate at full rate so the
            # first real matmuls run at 2.4 GHz instead of 1.2
            dw = consts.tile([128, 128], f32)
            nc.vector.memset(dw, 0.0)
            dx = consts.tile([128, CHUNK], f32)
            nc.vector.memset(dx, 0.0)
            for _ in range(7):
                dz = ops.tile([1, CHUNK], f32, tag="o_ps")
                nc.tensor.matmul(dz, dw[:, 0:1], dx, start=True, stop=True)

            # the c-matmul + out-copy of chunk j are emitted after chunk
            # j+1's W-matmuls so the PE never stalls on the min-chain
            pending = []

            def flush_pending():
                while pending:
                    mr_, orow_, nsl_ = pending.pop(0)
                    o_ps = ops.tile([1, CHUNK], f32, tag="o_ps")
                    nc.tensor.matmul(o_ps, c_sb, mr_, start=True, stop=True)
                    nc.scalar.copy(out=orow_[:, nsl_], in_=o_ps)

            n0 = 0
            gchunk = 0
            tot_chunks = N_PER_CORE // CHUNK
            for sc, sup in enumerate(SUPERS):
                x_sb = xin.tile([128, 2, sup], f32r, tag="x_sb")
                if sc == 0:
                    # split by d-chunk: lets the very first matmuls start
                    # after half the transfer
                    for dc in range(2):
                        nc.sync.dma_start(
                            out=x_sb[:, dc],
                            in_=xt[dc, :, n0:n0 + sup])
                else:
                    nc.sync.dma_start(
                        out=x_sb,
                        in_=xt[:, :, n0:n0 + sup].rearrange(
                            "dc d n -> d dc n"),
                    )
                orow = orows.tile([1, sup], f32, tag="orow")

                for j in range(sup // CHUNK):
                    # pair-chain (lower latency) for the final two chunks
                    # so the kernel-tail epilogue drain is short
                    is_last = gchunk >= tot_chunks - 2
                    gchunk += 1
                    nsl = slice(j * CHUNK, (j + 1) * CHUNK)
                    pst = []
                    for p in range(P):
                        z = ps.tile([128, CHUNK], f32, tag="z")
                        nc.tensor.matmul(z, wt_sb[:, 0, p, :], x_sb[:, 0, nsl],
                                         start=True, stop=False)
                        nc.tensor.matmul(z, wt_sb[:, 1, p, :], x_sb[:, 1, nsl],
                                         start=False, stop=True)
                        pst.append(z)

                    # min_p (z_p + b_p): ACT seeds with a bias-add copy out
                    # of PSUM, DVE folds each remaining PSUM tile via
                    # (ps + b) min acc; relu once at the end (commutes with
                    # min).  The last chunk uses two independent pair-chains
                    # instead -- lower latency, shorter kernel tail.
                    a0 = mins.tile([128, CHUNK], f32, tag="a0")
                    nc.scalar.activation(out=a0, in_=pst[0], func=Act.Identity,
                                         bias=b_sb[:, 0:1])
                    a1 = mins.tile([128, CHUNK], f32, tag="a1")
                    nc.vector.scalar_tensor_tensor(
                        out=a1, in0=pst[1], scalar=b_sb[:, 1:2], in1=a0,
                        op0=Alu.add, op1=Alu.min)
                    if not is_last:
                        a2 = mins.tile([128, CHUNK], f32, tag="a2")
                        nc.vector.scalar_tensor_tensor(
                            out=a2, in0=pst[2], scalar=b_sb[:, 2:3], in1=a1,
                            op0=Alu.add, op1=Alu.min)
                        a3 = mins.tile([128, CHUNK], f32, tag="a3")
                        nc.vector.scalar_tensor_tensor(
                            out=a3, in0=pst[3], scalar=b_sb[:, 3:4], in1=a2,
                            op0=Alu.add, op1=Alu.min)
                        mr = mins.tile([128, CHUNK], f32r, tag="mr")
                        nc.scalar.activation(out=mr, in_=a3, func=Act.Relu)
                    else:
                        a2 = mins.tile([128, CHUNK], f32, tag="a2")
                        nc.scalar.activation(out=a2, in_=pst[2],
                                             func=Act.Identity,
                                             bias=b_sb[:, 2:3])
                        a3 = mins.tile([128, CHUNK], f32, tag="a3")
                        nc.vector.scalar_tensor_tensor(
                            out=a3, in0=pst[3], scalar=b_sb[:, 3:4], in1=a2,
                            op0=Alu.add, op1=Alu.min)
                        ff = mins.tile([128, CHUNK], f32, tag="ff")
                        nc.vector.tensor_tensor(out=ff, in0=a1, in1=a3,
                                                op=Alu.min)
                        mr = mins.tile([128, CHUNK], f32r, tag="mr")
                        nc.scalar.activation(out=mr, in_=ff, func=Act.Relu)

                    pending.append((mr, orow, nsl))
                    flush_pending()

                # store via SWDGE (Pool queue): its sem wait must not block
                # later x-load triggers in the SP HWDGE FIFO
                nc.gpsimd.dma_start(out=out[:, n0:n0 + sup], in_=orow)
                n0 += sup

    nc.compile()
    return nc


def _get_compiled():
    global _compiled
    if _compiled is None:
        _compiled = _build_program()
    return _compiled


def _host_prep(x, weights, biases, c, indices):
    x = np.ascontiguousarray(np.asarray(x, dtype=np.float32))
    weights = np.asarray(weights, dtype=np.float32)
    biases = np.asarray(biases, dtype=np.float32)
    c = np.asarray(c, dtype=np.float32)
    indices = np.asarray(indices)

    # W_dense[d, r*P+p] = sum_{k: idx[r,k]==d} w[r,p,k]
    w_dense = np.zeros((D, R * P), dtype=np.float32)
    r_idx = np.broadcast_to(np.arange(R)[:, None, None], (R, P, K))
    p_idx = np.broadcast_to(np.arange(P)[None, :, None], (R, P, K))
    d_idx = np.broadcast_to(indices[:, None, :], (R, P, K))
    np.add.at(w_dense, (d_idx.ravel(), (r_idx * P + p_idx).ravel()),
              weights.ravel())

    # [128 d, 2 dc, 4 p, 128 r]
    wts = np.ascontiguousarray(
        w_dense.reshape(2, 128, R, P).transpose(1, 0, 3, 2))

    bia = np.ascontiguousarray(biases)      # [128 r, 4 p]
    cl = np.ascontiguousarray(c.reshape(128, 1))

    xt_pad = np.zeros((D, N_CORES * N_PER_CORE), dtype=np.float32)
    # place each core's 12500 rows at its padded offset
    xsplit = x.T.reshape(D, N_CORES, N // N_CORES)
    xt_pad_v = xt_pad.reshape(D, N_CORES, N_PER_CORE)
    xt_pad_v[:, :, :N // N_CORES] = xsplit

    in_maps = []
    for core in range(N_CORES):
        xt_c = np.ascontiguousarray(
            xt_pad_v[:, core, :].reshape(2, 128, N_PER_CORE))
        in_maps.append({"xt": xt_c, "wts": wts, "bia": bia, "cl": cl})
    return in_maps


def kernel(x, weights, biases, c, indices, _trace=False):
    from concourse.bass_utils import run_bass_kernel_spmd

    nc = _get_compiled()
    in_maps = _host_prep(x, weights, biases, c, indices)
    res = run_bass_kernel_spmd(nc, in_maps, core_ids=list(range(N_CORES)),
                               trace=_trace)
    per_core = np.stack([r["out"].reshape(N_PER_CORE)[:N // N_CORES]
                         for r in res.results])
    out = per_core.reshape(N).astype(np.float32)[:, None]
    if _trace:
        kernel.last_exec_time_ns = res.exec_time_ns
        kernel.last_results = res
    return out
